# revision 33
# baseline (speedup 1.0000x reference)
"""Trainium2 Bass kernel for nn_DiagonalTraining (ragged per-anti-diagonal linear).

Math (reference): for each batch image x[b] (SxS) and each anti-diagonal
i (elements x[b, r, i-r], r=0..i), apply a per-diagonal linear layer:
  out[b,i,q] = sum_{r<=i} x[b,r,i-r] * W[i,q,r] + bias[i,q]   (q <= i)
and scatter back: y[b,q,i-q] = out[b,i,q]; positions with r+c >= S keep x.

Distribution: diagonal i -> core i%8, slot j=i//8 (64 slots per core,
balanced by construction). Host packs, per (core, slot), an augmented
matrix whose rows are the contraction axis r:
  [ D^T | V ]  with D^T[r,b]=x[b,r,i-r], V[r,q]=W[i,q,r]  (r,q < ni=i+1)
zero-padded to a core-independent size NJ=8*(j+1) (>= ni for every
core) so the SPMD program is identical on all cores. The per-diagonal
bias is added on the host while scattering results back (elementwise,
~0.05% of the FLOPs; the whole einsum runs on device).

Device ("window streaming"): each slot is split into row-chunks padded
to 128 rows; chunk columns ([128, 32+NJ] blocks) are packed first-fit
into uniform [128, WF] window tiles. The windows are loaded by ~18
identical big SWDGE DMAs (128 descriptors of WF*4 bytes each) — full
128-partition DMAs spread evenly over all 16 SDMA engines and stream
at near-HBM rate, fully decoupled from compute. Matmuls read chunks at
static (window, column) offsets, accumulating psum[32, NJ] per slot
inside a bank-packed 4-slot group psum tile; one DVE copy per group
stages results, and all group stores run at the end of the SWDGE queue.

Only the live (lower-triangular) part of W is shipped/read (~29 MB/core
vs 512 MB full W) — the kernel is HBM-bound on ~those bytes.
float32r matmul operands: full 32-bit data, 1 cycle/column at N>=256.
"""

import sys

for _p in ("/opt/trn_rl_repo", "/opt/pypackages"):
    if _p not in sys.path:
        sys.path.append(_p)

import numpy as np

import concourse.bass as bass  # noqa: F401
import concourse.tile as tile
from concourse import bacc, mybir
from concourse.bass_utils import run_bass_kernel_spmd

B = 32          # batch
S = 512         # seq len / number of diagonals
N_CORES = 8
N_SLOTS = S // N_CORES  # 64 slots per core
DCOL = B        # width of the D^T block (batch on matmul M axis)
GROUP = 2       # slots per psum group
N_GROUPS = N_SLOTS // GROUP
WF = 3072       # window free size (f32 elems per partition) = 12 KiB descs

KCFG = {
    "compute": "f32r",  # "f32" | "f32r" | "bf16"
    "win_bufs": 9,
    "psum_bufs": 4,
}

# ---- static layout ----------------------------------------------------
# processing order: largest slot first
_ORDER = list(range(N_SLOTS - 1, -1, -1))
_GROUPS = [_ORDER[g * GROUP : (g + 1) * GROUP] for g in range(N_GROUPS)]


# window capacity: small first windows so the first matmuls start early
def _wcap(w):
    return (1024, 2048)[w] if w < 2 else WF


# chunk placement: every chunk gets its own [128, wd] column block
# (rows < 128 in the last chunk of a slot ship zero padding)
_SLOT_CHUNKS = {}   # j -> list of (win, cbase, pbase, rows, row_start)
_cur_win = 0
_cur_col = 0


def _new_block(wd):
    global _cur_win, _cur_col
    if _cur_col + wd > _wcap(_cur_win):
        _cur_win += 1
        _cur_col = 0
    blk = (_cur_win, _cur_col)
    _cur_col += wd
    return blk


for _j in _ORDER:
    _NJ = 8 * (_j + 1)
    _wd = DCOL + _NJ
    _s = -(-_NJ // 128)
    _chs = []
    for _c in range(_s):
        _rows = min(128, _NJ - 128 * _c)
        _w, _cb = _new_block(_wd)
        _chs.append((_w, _cb, 0, _rows, 128 * _c))
    _SLOT_CHUNKS[_j] = _chs
N_WINS = _cur_win + 1
# exact used width per window (ship no window-tail padding)
_WIN_W = [0] * N_WINS
for _j, _chs in _SLOT_CHUNKS.items():
    _wd = DCOL + 8 * (_j + 1)
    for _w, _cb, _pb, _rows, _rs in _chs:
        _WIN_W[_w] = max(_WIN_W[_w], _cb + _wd)
_WIN_OFF = []
_boff = 0
for _w in range(N_WINS):
    _WIN_OFF.append(_boff)
    _boff += 128 * _WIN_W[_w]
BLOB_ELEMS = _boff

# psum group column layout (bank-aligned, no matmul straddles a bank)
_BANK = 512
_GROUP_COLS = []
_GROUP_W = []
for _slots in _GROUPS:
    _col = 0
    _cols = []
    for _j in _slots:
        _NJ = 8 * (_j + 1)
        if _col // _BANK != (_col + _NJ - 1) // _BANK:
            _col = ((_col + _BANK - 1) // _BANK) * _BANK
        _cols.append((_j, _col))
        _col += _NJ
    _GROUP_COLS.append(_cols)
    _GROUP_W.append(_col)

_GOUT_OFF = []
_SLOT_OUT = {}
_goff = 0
for _g in range(N_GROUPS):
    _GOUT_OFF.append(_goff)
    for _j, _col in _GROUP_COLS[_g]:
        _SLOT_OUT[_j] = (_g, _col)
    _goff += B * _GROUP_W[_g]
OUT_ELEMS = _goff

_compiled_nc = None


def _build_program():
    global _compiled_nc
    if _compiled_nc is not None:
        return _compiled_nc

    from contextlib import ExitStack

    nc = bacc.Bacc("TRN2", target_bir_lowering=False, debug=False)
    f32 = mybir.dt.float32
    mm_dt = {
        "f32": f32,
        "f32r": mybir.dt.float32r,
        "bf16": mybir.dt.bfloat16,
    }[KCFG["compute"]]
    blob = nc.dram_tensor("blob", [BLOB_ELEMS], mm_dt, kind="ExternalInput").ap()
    outb = nc.dram_tensor("outblob", [OUT_ELEMS], f32, kind="ExternalOutput").ap()

    with tile.TileContext(nc) as tc, ExitStack() as ctx:
        win_pool = ctx.enter_context(
            tc.tile_pool(name="win", bufs=KCFG["win_bufs"])
        )
        acc_pool = ctx.enter_context(tc.tile_pool(name="acc", bufs=1))
        psum_pool = ctx.enter_context(
            tc.tile_pool(name="psum", bufs=KCFG["psum_bufs"], space="PSUM")
        )

        # window tiles are loaded lazily in program order; keep handles
        win_tiles = [None] * N_WINS

        def ensure_win(w):
            if win_tiles[w] is None:
                wf = _WIN_W[w]
                t = win_pool.tile([128, wf], mm_dt)
                src = blob[_WIN_OFF[w] : _WIN_OFF[w] + 128 * wf].rearrange(
                    "(p f) -> p f", p=128, f=wf
                )
                nc.gpsimd.dma_start(t[:], src)
                win_tiles[w] = t
            return win_tiles[w]

        tot_w = OUT_ELEMS // B
        acc_t = acc_pool.tile([B, tot_w], f32)
        for g, slots in enumerate(_GROUPS):
            gw = _GROUP_W[g]
            gcol = _GOUT_OFF[g] // B
            psum_t = psum_pool.tile([B, gw], f32)
            for j, col in _GROUP_COLS[g]:
                NJ = 8 * (j + 1)
                wd = DCOL + NJ
                chs = _SLOT_CHUNKS[j]
                for c, (w, cb, pb, rows, _rs) in enumerate(chs):
                    t = ensure_win(w)
                    nc.tensor.matmul(
                        psum_t[:, col : col + NJ],
                        t[pb : pb + rows, cb : cb + DCOL],
                        t[pb : pb + rows, cb + DCOL : cb + wd],
                        start=(c == 0),
                        stop=(c == len(chs) - 1),
                    )
            nc.vector.tensor_copy(acc_t[:, gcol : gcol + gw], psum_t[:])
        # staged stores: earlier group ranges flush while later compute
        # still runs. All after the loads on the Pool queue, so a store
        # wait only ever blocks later (even more dependent) stores.
        dstv = outb[:].rearrange("(p w) -> p w", p=B, w=tot_w)
        cuts = [0, _GOUT_OFF[16] // B, _GOUT_OFF[26] // B, tot_w]
        for a, bnd in zip(cuts, cuts[1:]):
            nc.gpsimd.dma_start(dstv[:, a:bnd], acc_t[:, a:bnd])

    nc.compile()
    _compiled_nc = nc
    return nc


def _pack_core(k, x, W, bias):
    np_dt = np.float32
    if KCFG["compute"] == "bf16":
        import ml_dtypes

        np_dt = ml_dtypes.bfloat16
    blob = np.zeros(BLOB_ELEMS, np_dt)
    bw = blob.reshape(128, -1) if False else None  # noqa
    # windows: [128, WF] images laid out window-major
    for j in range(N_SLOTS):
        i = N_CORES * j + k
        ni = i + 1
        NJ = 8 * (j + 1)
        wd = DCOL + NJ
        M = np.zeros((NJ, wd), np.float32)
        r = np.arange(ni)
        M[:ni, :DCOL] = x[:, r, i - r].T               # D^T[r, b]
        M[:ni, DCOL : DCOL + ni] = W[i, :ni, :ni].T    # V[r, q]
        for w, cb, pb, rows, rs in _SLOT_CHUNKS[j]:
            rl = M[rs : rs + rows]                     # [rows, wd]
            wf = _WIN_W[w]
            img = blob[_WIN_OFF[w] : _WIN_OFF[w] + 128 * wf].reshape(128, wf)
            img[pb : pb + rows, cb : cb + wd] = rl.astype(np_dt)
    return blob


def kernel(x, W, b):
    x = np.asarray(x, np.float32)
    W = np.asarray(W, np.float32)
    b = np.asarray(b, np.float32)

    nc = _build_program()
    in_maps = [{"blob": _pack_core(k, x, W, b)} for k in range(N_CORES)]
    res = run_bass_kernel_spmd(nc, in_maps, list(range(N_CORES)))

    y = x.copy()
    tot_w = OUT_ELEMS // B
    for k in range(N_CORES):
        ob = res.results[k]["outblob"].reshape(B, tot_w)
        for j in range(N_SLOTS):
            i = N_CORES * j + k
            ni = i + 1
            g, col = _SLOT_OUT[j]
            gcol = _GOUT_OFF[g] // B + col
            q = np.arange(ni)
            y[:, q, i - q] = ob[:, gcol : gcol + ni] + b[i, :ni][None]
    return y


# revision 34
# speedup vs baseline: 1.1416x; 1.1416x over previous
"""Trainium2 Bass kernel for nn_DiagonalTraining (ragged per-anti-diagonal linear).

Math (reference): for each batch image x[b] (SxS) and each anti-diagonal
i (elements x[b, r, i-r], r=0..i), apply a per-diagonal linear layer:
  out[b,i,q] = sum_{r<=i} x[b,r,i-r] * W[i,q,r] + bias[i,q]   (q <= i)
and scatter back: y[b,q,i-q] = out[b,i,q]; positions with r+c >= S keep x.

Distribution: diagonal i -> core i%8, slot j=i//8 (64 slots per core,
balanced by construction). Host packs, per (core, slot), an augmented
matrix whose rows are the contraction axis r:
  [ D^T | V ]  with D^T[r,b]=x[b,r,i-r], V[r,q]=W[i,q,r]  (r,q < ni=i+1)
zero-padded to a core-independent size NJ=8*(j+1) (>= ni for every
core) so the SPMD program is identical on all cores. The per-diagonal
bias is added on the host while scattering results back (elementwise,
~0.05% of the FLOPs; the whole einsum runs on device).

Device ("window streaming"): each slot is split into row-chunks padded
to 128 rows; chunk columns ([128, 32+NJ] blocks) are packed first-fit
into uniform [128, WF] window tiles. The windows are loaded by ~18
identical big SWDGE DMAs (128 descriptors of WF*4 bytes each) — full
128-partition DMAs spread evenly over all 16 SDMA engines and stream
at near-HBM rate, fully decoupled from compute. Matmuls read chunks at
static (window, column) offsets, accumulating psum[32, NJ] per slot
inside a bank-packed 4-slot group psum tile; one DVE copy per group
stages results, and all group stores run at the end of the SWDGE queue.

Only the live (lower-triangular) part of W is shipped/read (~29 MB/core
vs 512 MB full W) — the kernel is HBM-bound on ~those bytes.
float32r matmul operands: full 32-bit data, 1 cycle/column at N>=256.
"""

import sys

for _p in ("/opt/trn_rl_repo", "/opt/pypackages"):
    if _p not in sys.path:
        sys.path.append(_p)

import numpy as np

import concourse.bass as bass  # noqa: F401
import concourse.tile as tile
from concourse import bacc, mybir
from concourse.bass_utils import run_bass_kernel_spmd

B = 32          # batch
S = 512         # seq len / number of diagonals
N_CORES = 8
N_SLOTS = S // N_CORES  # 64 slots per core
DCOL = B        # width of the D^T block (batch on matmul M axis)
GROUP = 4       # slots per psum group
N_GROUPS = N_SLOTS // GROUP
WF = 3072       # window free size (f32 elems per partition) = 12 KiB descs

KCFG = {
    "compute": "f32r",  # "f32" | "f32r" | "bf16"
    "win_bufs": 9,
    "psum_bufs": 2,
}

# ---- static layout ----------------------------------------------------
# processing order: largest slot first
_ORDER = list(range(N_SLOTS - 1, -1, -1))
_GROUPS = [_ORDER[g * GROUP : (g + 1) * GROUP] for g in range(N_GROUPS)]


# window capacity: small first windows so the first matmuls start early
def _wcap(w):
    return (1024, 2048)[w] if w < 2 else WF


# chunk placement: every chunk gets its own [128, wd] column block
# (rows < 128 in the last chunk of a slot ship zero padding)
_SLOT_CHUNKS = {}   # j -> list of (win, cbase, pbase, rows, row_start)
_cur_win = 0
_cur_col = 0


def _new_block(wd):
    global _cur_win, _cur_col
    if _cur_col + wd > _wcap(_cur_win):
        _cur_win += 1
        _cur_col = 0
    blk = (_cur_win, _cur_col)
    _cur_col += wd
    return blk


for _j in _ORDER:
    _NJ = 8 * (_j + 1)
    _wd = DCOL + _NJ
    _s = -(-_NJ // 128)
    _chs = []
    for _c in range(_s):
        _rows = min(128, _NJ - 128 * _c)
        _w, _cb = _new_block(_wd)
        _chs.append((_w, _cb, 0, _rows, 128 * _c))
    _SLOT_CHUNKS[_j] = _chs
N_WINS = _cur_win + 1
# exact used width per window (ship no window-tail padding)
_WIN_W = [0] * N_WINS
for _j, _chs in _SLOT_CHUNKS.items():
    _wd = DCOL + 8 * (_j + 1)
    for _w, _cb, _pb, _rows, _rs in _chs:
        _WIN_W[_w] = max(_WIN_W[_w], _cb + _wd)
_WIN_OFF = []
_boff = 0
for _w in range(N_WINS):
    _WIN_OFF.append(_boff)
    _boff += 128 * _WIN_W[_w]
BLOB_ELEMS = _boff

# psum group column layout (bank-aligned, no matmul straddles a bank)
_BANK = 512
_GROUP_COLS = []
_GROUP_W = []
for _slots in _GROUPS:
    _col = 0
    _cols = []
    for _j in _slots:
        _NJ = 8 * (_j + 1)
        if _col // _BANK != (_col + _NJ - 1) // _BANK:
            _col = ((_col + _BANK - 1) // _BANK) * _BANK
        _cols.append((_j, _col))
        _col += _NJ
    _GROUP_COLS.append(_cols)
    _GROUP_W.append(_col)

_GOUT_OFF = []
_SLOT_OUT = {}
_goff = 0
for _g in range(N_GROUPS):
    _GOUT_OFF.append(_goff)
    for _j, _col in _GROUP_COLS[_g]:
        _SLOT_OUT[_j] = (_g, _col)
    _goff += B * _GROUP_W[_g]
OUT_ELEMS = _goff

_compiled_nc = None


def _build_program():
    global _compiled_nc
    if _compiled_nc is not None:
        return _compiled_nc

    from contextlib import ExitStack

    nc = bacc.Bacc("TRN2", target_bir_lowering=False, debug=False)
    f32 = mybir.dt.float32
    mm_dt = {
        "f32": f32,
        "f32r": mybir.dt.float32r,
        "bf16": mybir.dt.bfloat16,
    }[KCFG["compute"]]
    blob = nc.dram_tensor("blob", [BLOB_ELEMS], mm_dt, kind="ExternalInput").ap()
    outb = nc.dram_tensor("outblob", [OUT_ELEMS], f32, kind="ExternalOutput").ap()

    with tile.TileContext(nc) as tc, ExitStack() as ctx:
        win_pool = ctx.enter_context(
            tc.tile_pool(name="win", bufs=KCFG["win_bufs"])
        )
        acc_pool = ctx.enter_context(tc.tile_pool(name="acc", bufs=1))
        psum_pool = ctx.enter_context(
            tc.tile_pool(name="psum", bufs=KCFG["psum_bufs"], space="PSUM")
        )

        # window tiles are loaded lazily in program order; keep handles
        win_tiles = [None] * N_WINS

        def ensure_win(w):
            if win_tiles[w] is None:
                wf = _WIN_W[w]
                t = win_pool.tile([128, wf], mm_dt)
                src = blob[_WIN_OFF[w] : _WIN_OFF[w] + 128 * wf].rearrange(
                    "(p f) -> p f", p=128, f=wf
                )
                nc.gpsimd.dma_start(t[:], src)
                win_tiles[w] = t
            return win_tiles[w]

        tot_w = OUT_ELEMS // B
        acc_t = acc_pool.tile([B, tot_w], f32)
        for g, slots in enumerate(_GROUPS):
            gw = _GROUP_W[g]
            gcol = _GOUT_OFF[g] // B
            psum_t = psum_pool.tile([B, gw], f32)
            for j, col in _GROUP_COLS[g]:
                NJ = 8 * (j + 1)
                wd = DCOL + NJ
                chs = _SLOT_CHUNKS[j]
                for c, (w, cb, pb, rows, _rs) in enumerate(chs):
                    t = ensure_win(w)
                    nc.tensor.matmul(
                        psum_t[:, col : col + NJ],
                        t[pb : pb + rows, cb : cb + DCOL],
                        t[pb : pb + rows, cb + DCOL : cb + wd],
                        start=(c == 0),
                        stop=(c == len(chs) - 1),
                    )
            nc.vector.tensor_copy(acc_t[:, gcol : gcol + gw], psum_t[:])
        # staged stores: earlier group ranges flush while later compute
        # still runs. All after the loads on the Pool queue, so a store
        # wait only ever blocks later (even more dependent) stores.
        dstv = outb[:].rearrange("(p w) -> p w", p=B, w=tot_w)
        cuts = [0, _GOUT_OFF[8] // B, _GOUT_OFF[13] // B, tot_w]
        for a, bnd in zip(cuts, cuts[1:]):
            nc.gpsimd.dma_start(dstv[:, a:bnd], acc_t[:, a:bnd])

    nc.compile()
    _compiled_nc = nc
    return nc


def _pack_core(k, x, W, bias):
    np_dt = np.float32
    if KCFG["compute"] == "bf16":
        import ml_dtypes

        np_dt = ml_dtypes.bfloat16
    blob = np.zeros(BLOB_ELEMS, np_dt)
    bw = blob.reshape(128, -1) if False else None  # noqa
    # windows: [128, WF] images laid out window-major
    for j in range(N_SLOTS):
        i = N_CORES * j + k
        ni = i + 1
        NJ = 8 * (j + 1)
        wd = DCOL + NJ
        M = np.zeros((NJ, wd), np.float32)
        r = np.arange(ni)
        M[:ni, :DCOL] = x[:, r, i - r].T               # D^T[r, b]
        M[:ni, DCOL : DCOL + ni] = W[i, :ni, :ni].T    # V[r, q]
        for w, cb, pb, rows, rs in _SLOT_CHUNKS[j]:
            rl = M[rs : rs + rows]                     # [rows, wd]
            wf = _WIN_W[w]
            img = blob[_WIN_OFF[w] : _WIN_OFF[w] + 128 * wf].reshape(128, wf)
            img[pb : pb + rows, cb : cb + wd] = rl.astype(np_dt)
    return blob


def kernel(x, W, b):
    x = np.asarray(x, np.float32)
    W = np.asarray(W, np.float32)
    b = np.asarray(b, np.float32)

    nc = _build_program()
    in_maps = [{"blob": _pack_core(k, x, W, b)} for k in range(N_CORES)]
    res = run_bass_kernel_spmd(nc, in_maps, list(range(N_CORES)))

    y = x.copy()
    tot_w = OUT_ELEMS // B
    for k in range(N_CORES):
        ob = res.results[k]["outblob"].reshape(B, tot_w)
        for j in range(N_SLOTS):
            i = N_CORES * j + k
            ni = i + 1
            g, col = _SLOT_OUT[j]
            gcol = _GOUT_OFF[g] // B + col
            q = np.arange(ni)
            y[:, q, i - q] = ob[:, gcol : gcol + ni] + b[i, :ni][None]
    return y
